# revision 14
# baseline (speedup 1.0000x reference)
"""Trainium2 Bass kernel for nn_ContrastiveLoss (SCAN text-to-image loss).

Full inputs in, full (scalar) output out. Captions are sharded across 8
NeuronCores (16 captions each, images replicated). Each core computes its
scores[:, c_slice] block on device; the host gathers the 8 slices and
computes the tiny [128,128] diagonal-margin loss in numpy.

Math notes (exact reductions of the reference):
  - softmax over regions needs no normalizer: with E = exp(9 * a_norm),
    cos = (sum_r E*A) / (||cap|| * sqrt(E^T G E)) since the softmax
    normalizer Z cancels between numerator and denominator.
  - wei-norm uses the per-image Gram matrix G_i = X_i X_i^T; the
    blockdiag-masked 3-image Gram blocks are host-precomputed (they are
    pure input preprocessing, like the transposed/padded image layout).
  - word masking is folded into the caption operand host-side; caption
    word norms ||cap_w||^2 (cn2) are host-precomputed as well.
  - all matmul operands are bf16 (validated: end-to-end rel err ~5e-5);
    PSUM accumulation stays fp32.
  - the raw attention A is consumed immediately (Prelu -> B, inverse
    Prelu B -> A2 = min(10B, B)), so each attention PSUM bank is freed
    after one ACT pass and the pipeline can run 3 groups deep per pool.
"""

import numpy as np

# Problem geometry (hardcoded per contract).
I, R, D, W = 128, 36, 512, 24
NCORES = 8
CS = I // NCORES          # captions per core = 16
GI = 3                    # images per PE group (3*36 = 108 <= 128 partitions)
GR = GI * R               # 108
NG = (I + GI - 1) // GI   # 43 groups
SW = 128                  # stationary width: pad each group's imT slice to
                          # 128 columns so LDWEIGHTS takes the FWL fast path
IRP = (NG - 1) * GR + SW  # 4664 padded image-region columns
CW = CS * W               # 384 caption-word columns per core
NK = D // 128             # 4 contraction chunks
LN9 = float(np.log(9.0))

_CACHE: dict = {}


def _build_program(reps: int = 1, with_loss: bool = True):
    import concourse.bacc as bacc
    import concourse.mybir as mybir
    import concourse.tile as tile

    f32 = mybir.dt.float32
    bf16 = mybir.dt.bfloat16
    Act = mybir.ActivationFunctionType
    Alu = mybir.AluOpType
    X = mybir.AxisListType.X

    nc = bacc.Bacc("TRN2", target_bir_lowering=False, debug=False,
                   num_devices=NCORES)

    imT_d = nc.dram_tensor("imT", [NK, 128, IRP], bf16, kind="ExternalInput")
    capT_d = nc.dram_tensor("capT", [NK, 128, CW], bf16, kind="ExternalInput")
    gsb_d = nc.dram_tensor("gsb", [GR, NG * GR], bf16, kind="ExternalInput")
    cn2_d = nc.dram_tensor("cn2", [128, CW], f32, kind="ExternalInput")
    wmask_d = nc.dram_tensor("wmask", [128, CW], f32, kind="ExternalInput")
    inds_d = nc.dram_tensor("inds", [GR, NG * 128], bf16, kind="ExternalInput")
    sc_d = nc.dram_tensor("scores", [128, CS], f32, kind="ExternalOutput")

    with tile.TileContext(nc) as tc:
        with (
            tc.tile_pool(name="const", bufs=1) as cp,
            tc.tile_pool(name="work", bufs=6) as wp,
            tc.tile_pool(name="small", bufs=8) as sp,
        ):
            # ---- constants -------------------------------------------------
            capT = [cp.tile([128, CW], bf16, tag=f"capT{k}", name=f"capT{k}")
                    for k in range(NK)]
            gsb = cp.tile([GR, NG * GR], bf16, tag="gsb")
            cn2 = cp.tile([128, CW], f32, tag="cn2")
            wmask = cp.tile([128, CW], f32, tag="wmask")
            inds = cp.tile([GR, NG * 128], bf16, tag="inds")
            scores = cp.tile([128, CS], f32, tag="scores")
            ln9c = cp.tile([128, 1], f32, tag="ln9c")
            nc.gpsimd.memset(ln9c[:], LN9)
            imTsb = [cp.tile([128, IRP], bf16, tag=f"imT{k}", name=f"imT{k}")
                     for k in range(NK)]

            for k in range(NK):
                nc.sync.dma_start(out=capT[k][:], in_=capT_d[k])
            nc.sync.dma_start(out=cn2[:], in_=cn2_d[:])
            nc.sync.dma_start(out=wmask[:], in_=wmask_d[:])
            nc.sync.dma_start(out=inds[:], in_=inds_d[:])
            nc.sync.dma_start(out=gsb[:], in_=gsb_d[:])
            # images resident in SBUF; column-block DMAs (~7 groups each),
            # all 4 contraction chunks of a column block before the next
            # block, so the first groups unblock as early as possible
            CB = 7 * GR
            for c0 in range(0, IRP, CB):
                c1 = min(c0 + CB, IRP)
                for k in range(NK):
                    nc.sync.dma_start(
                        out=imTsb[k][:, c0:c1],
                        in_=imT_d[k, :, c0:c1],
                    )

            NP = NG // 2  # group pairs; NG=43 is odd -> last pair is single
            for rep in range(reps):
                with tc.tile_pool(name=f"ps_acc{rep}", bufs=1,
                                  space="PSUM") as pa:
                    ps_ne = pa.tile([128, CW], f32, tag="ne", name="ps_ne")
                    ps_q = pa.tile([128, CW], f32, tag="q", name="ps_q")

                    with (
                        tc.tile_pool(name=f"ps_a{rep}", bufs=2,
                                     space="PSUM") as pta,
                        tc.tile_pool(name=f"ps_g{rep}", bufs=1,
                                     space="PSUM") as ptg,
                    ):
                      n2q = None
                      n2_off = 0
                      pend = []
                      for p in range(NP + 1):
                        gs = [2 * p] if p == NP else [2 * p, 2 * p + 1]
                        U = len(gs)
                        UW = U * CW
                        if p % 2 == 0:
                            # n2 of two consecutive pairs shares one tile so
                            # the small Ln/Exp ACT ops run once per 4 groups
                            n2q = sp.tile([GR, 4 * CS], f32, tag="n2",
                                          name="n2q")
                            n2_off = 0
                        # 2-bank PSUM tile: A(g0) [0:384], A(g1) [512:896]
                        pa2 = pta.tile([GR, 512 * U], f32, tag="a",
                                       name="pa2")
                        for u, g in enumerate(gs):
                            for k in range(NK):
                                sl = imTsb[k][:, g * GR:(g + 1) * GR]
                                nc.tensor.matmul(
                                    pa2[:, 512 * u:512 * u + CW],
                                    sl, capT[k][:],
                                    start=(k == 0), stop=(k == NK - 1))
                        av = (pa2[:].rearrange("p (u w) -> p u w", u=U)
                              [:, :, 0:CW])
                        # B = leaky_relu(A), one batched ACT pass per pair;
                        # frees both attention PSUM banks
                        B = wp.tile([GR, UW], bf16, tag="B", name="B")
                        nc.scalar.activation(
                            B[:].rearrange("p (u w) -> p u w", u=U), av,
                            Act.Prelu, alpha=0.1)
                        # A2 = min(10B, B) inverts the leaky-relu (DVE: the
                        # backend rejects TensorScalarPtr on Pool)
                        A2 = wp.tile([GR, UW], bf16, tag="A2", name="A2")
                        nc.vector.scalar_tensor_tensor(
                            A2[:], B[:], 10.0, B[:], Alu.mult, Alu.min)
                        # n2 = sum_w B^2 per (row, caption); the W-reduce is
                        # split as a bf16 2x-mode add of word halves plus a
                        # half-width reduce (cheaper than one 1x reduce)
                        B2 = wp.tile([GR, UW], bf16, tag="B2", name="B2")
                        nc.gpsimd.tensor_tensor(B2[:], B[:], B[:], Alu.mult)
                        B2v = B2[:].rearrange("p (c w) -> p c w", w=W)
                        B2h = wp.tile([GR, UW // 2], bf16, tag="B2h",
                                      name="B2h")
                        nc.vector.tensor_tensor(
                            B2h[:].rearrange("p (c w) -> p c w", w=W // 2),
                            B2v[:, :, 0:W // 2],
                            B2v[:, :, W // 2:W],
                            Alu.add,
                        )
                        nc.vector.reduce_sum(
                            n2q[:, n2_off:n2_off + U * CS],
                            B2h[:].rearrange("p (c w) -> p c w", w=W // 2),
                            axis=X,
                        )
                        pend.append((B, A2, n2_off, U, gs))
                        n2_off += U * CS
                        if p % 2 == 0 and p != NP:
                            continue
                        # rinv9 = 9 * n2^(-1/2) via exp(-0.5*ln + ln9), one
                        # small ACT pair per 4 groups
                        nf = n2_off
                        lnn = sp.tile([GR, 4 * CS], f32, tag="lnn",
                                      name="lnn")
                        nc.scalar.activation(lnn[:, 0:nf], n2q[:, 0:nf],
                                             Act.Ln)
                        rinv9 = sp.tile([GR, 4 * CS], bf16, tag="rinv9",
                                        name="rinv9")
                        nc.scalar.activation(rinv9[:, 0:nf], lnn[:, 0:nf],
                                             Act.Exp, scale=-0.5,
                                             bias=ln9c[0:GR])
                        for Bt, A2t, off, Ut, gst in pend:
                            UWt = Ut * CW
                            # Bn9 = 9 * B * rinv (broadcast over words)
                            Bn9 = wp.tile([GR, UWt], bf16, tag="Bn9",
                                          name="Bn9")
                            nc.gpsimd.tensor_tensor(
                                Bn9[:].rearrange("p (c w) -> p c w", w=W),
                                Bt[:].rearrange("p (c w) -> p c w", w=W),
                                rinv9[:, off:off + Ut * CS]
                                .rearrange("p (c u) -> p c u", u=1)
                                .broadcast_to((GR, Ut * CS, W)),
                                Alu.mult,
                            )
                            # E = exp(Bn9), one batched ACT pass per pair
                            E = wp.tile([GR, UWt], bf16, tag="E", name="E")
                            nc.scalar.activation(E[:], Bn9[:], Act.Exp)
                            # GE = blockdiag(G) @ E, into a 2-bank tile
                            pg2 = ptg.tile([GR, 512 * Ut], f32, tag="ge",
                                           name="pg2")
                            for u, g in enumerate(gst):
                                nc.tensor.matmul(
                                    pg2[:, 512 * u:512 * u + CW],
                                    gsb[:, g * GR:(g + 1) * GR],
                                    E[:, u * CW:(u + 1) * CW],
                                    start=True, stop=True)
                            # EA = E * A2 (Pool) / EGE = E * GE (DVE)
                            EA = wp.tile([GR, UWt], bf16, tag="EA",
                                         name="EA")
                            nc.gpsimd.tensor_tensor(EA[:], E[:], A2t[:],
                                                    Alu.mult)
                            EG = wp.tile([GR, UWt], bf16, tag="EG",
                                         name="EG")
                            nc.vector.tensor_tensor(
                                EG[:].rearrange("p (u w) -> p u w", u=Ut),
                                E[:].rearrange("p (u w) -> p u w", u=Ut),
                                pg2[:].rearrange("p (u w) -> p u w", u=Ut)
                                [:, :, 0:CW],
                                Alu.mult,
                            )
                            # block-sum regions into stacked accumulators
                            for u, g in enumerate(gst):
                                ind = inds[:, g * 128:(g + 1) * 128]
                                cw0, cw1 = u * CW, (u + 1) * CW
                                nc.tensor.matmul(
                                    ps_ne[:], ind, EA[:, cw0:cw1],
                                    start=(g == 0), stop=(g == NG - 1))
                                nc.tensor.matmul(
                                    ps_q[:], ind, EG[:, cw0:cw1],
                                    start=(g == 0), stop=(g == NG - 1))
                        pend = []

                    # ---- epilogue: cos -> logsumexp ------------------------
                    with tc.tile_pool(name=f"fin{rep}", bufs=1) as fp_:
                        qc = fp_.tile([128, CW], f32, tag="qc", name="qc")
                        nc.vector.tensor_tensor(qc[:], ps_q[:], cn2[:],
                                                Alu.mult)
                        lq = fp_.tile([128, CW], f32, tag="lq", name="lq")
                        nc.scalar.activation(lq[:], qc[:], Act.Ln)
                        rsq = fp_.tile([128, CW], f32, tag="rsq", name="rsq")
                        nc.scalar.activation(rsq[:], lq[:], Act.Exp,
                                             scale=-0.5)
                        cosm = fp_.tile([128, CW], f32, tag="cosm",
                                        name="cosm")
                        nc.vector.tensor_tensor(cosm[:], ps_ne[:], rsq[:],
                                                Alu.mult)
                        ex = fp_.tile([128, CW], f32, tag="ex", name="ex")
                        nc.scalar.activation(ex[:], cosm[:], Act.Exp,
                                             scale=6.0)
                        exm = fp_.tile([128, CW], f32, tag="exm", name="exm")
                        nc.vector.tensor_tensor(exm[:], ex[:], wmask[:],
                                                Alu.mult)
                        rs = fp_.tile([128, CS], f32, tag="rs", name="rs")
                        nc.vector.reduce_sum(
                            rs[:], exm[:].rearrange("p (c w) -> p c w", w=W),
                            axis=X,
                        )
                        # scores (x6): L = ln(sum) = 6 * row_sim
                        nc.scalar.activation(scores[:], rs[:], Act.Ln)
                        if rep == reps - 1:
                            nc.sync.dma_start(out=sc_d[:], in_=scores[:])

    # Pin activation-table selection to the one set that contains every
    # scalar-engine function we use (prelu, ln, exp): otherwise the
    # inserter alternates sets and pays a 1.3us table load per switch.
    from concourse import bacc as _bacc_mod
    _orig_tables = _bacc_mod.get_activation_tables

    def _pinned_tables(arch):
        t = _orig_tables(arch)
        keep = "natural_log_exp_and_others"
        return {k: (v if k == keep else set()) for k, v in t.items()}

    _bacc_mod.get_activation_tables = _pinned_tables
    try:
        nc.compile()
    finally:
        _bacc_mod.get_activation_tables = _orig_tables
    return nc


def _prep_in_maps(images, captions, cap_lens):
    import ml_dtypes

    bf = ml_dtypes.bfloat16
    images = np.ascontiguousarray(images, dtype=np.float32)
    captions = np.ascontiguousarray(captions, dtype=np.float32)
    cap_lens = np.asarray(cap_lens, dtype=np.int32)

    imt = images.transpose(2, 0, 1).reshape(D, I * R)
    # pad the ragged last group with a dummy (real-valued) region block
    imt_p = np.concatenate([imt, imt[:, : IRP - I * R]], axis=1)
    imt_p = np.ascontiguousarray(imt_p).astype(bf).reshape(NK, 128, IRP)

    # per-image Gram matrices from the bf16-rounded images (matches the
    # on-device attention operand rounding), assembled as the blockdiag
    # 3-image stationary blocks per group
    imb = images.astype(bf).astype(np.float32)          # [I, R, D]
    G = np.matmul(imb, imb.transpose(0, 2, 1))          # [I, R, R]
    gsb = np.zeros((GR, NG * GR), dtype=bf)
    for g in range(NG):
        for b in range(GI):
            m = GI * g + b
            if m >= I:
                m = 0  # dummy pad image (matches imt_p padding)
            gsb[b * R:(b + 1) * R,
                g * GR + b * R:g * GR + (b + 1) * R] = G[m]

    inds = np.zeros((GR, NG * 128), dtype=bf)
    for g in range(NG):
        for k in range(GR):
            m = GI * g + k // R
            if m < I:
                inds[k, g * 128 + m] = 1.0

    wvalid = (np.arange(W)[None, :] < cap_lens[:, None]).astype(np.float32)
    capm = captions * wvalid[:, :, None]              # masked words zeroed
    cn2_full = np.sum(captions * captions, axis=2)    # [I, W] unmasked norms

    in_maps = []
    for r in range(NCORES):
        cap = capm[r * CS:(r + 1) * CS]                  # [16, 24, 512]
        capT = np.ascontiguousarray(
            cap.transpose(2, 0, 1).reshape(D, CW)
        ).astype(bf).reshape(NK, 128, CW)
        wm = np.ascontiguousarray(np.broadcast_to(
            wvalid[r * CS:(r + 1) * CS].reshape(1, CW), (128, CW)
        )).astype(np.float32)
        cn2 = np.ascontiguousarray(np.broadcast_to(
            cn2_full[r * CS:(r + 1) * CS].reshape(1, CW), (128, CW)
        )).astype(np.float32)
        in_maps.append({
            "imT": imt_p,
            "capT": capT,
            "gsb": gsb,
            "cn2": cn2,
            "wmask": wm,
            "inds": inds,
        })
    return in_maps


def _get_nc(reps: int = 1, with_loss: bool = True):
    key = (reps, with_loss)
    if key not in _CACHE:
        _CACHE[key] = _build_program(reps, with_loss)
    return _CACHE[key]


def _host_loss(scores_x6: np.ndarray) -> np.float32:
    """scores_x6: [I, C] = 6 * row_sim. Diagonal-margin loss in fp32."""
    s = (scores_x6 / 6.0).astype(np.float32)
    diag = np.diag(s)
    cost_s = np.clip(0.2 + s - diag[:, None], 0.0, None)
    cost_im = np.clip(0.2 + s - diag[None, :], 0.0, None)
    np.fill_diagonal(cost_s, 0.0)
    np.fill_diagonal(cost_im, 0.0)
    return np.float32(cost_s.max(axis=1).sum() + cost_im.max(axis=0).sum())


def kernel(images, captions, cap_lens):
    from concourse.bass_utils import run_bass_kernel_spmd

    nc = _get_nc()
    in_maps = _prep_in_maps(images, captions, cap_lens)
    res = run_bass_kernel_spmd(nc, in_maps, core_ids=list(range(NCORES)))
    blocks = [np.asarray(res.results[r]["scores"]) for r in range(NCORES)]
    scores_x6 = np.concatenate(blocks, axis=1)        # [128, 128]
    return _host_loss(scores_x6)


# revision 15
# speedup vs baseline: 2.7370x; 2.7370x over previous
"""Trainium2 Bass kernel for nn_ContrastiveLoss (SCAN text-to-image loss).

Full inputs in, full (scalar) output out. Captions are sharded across 8
NeuronCores (16 captions each, images replicated). Each core computes its
scores[:, c_slice] block on device; the host gathers the 8 slices and
computes the tiny [128,128] diagonal-margin loss in numpy.

Math notes (exact reductions of the reference):
  - softmax over regions needs no normalizer: with E = exp(9 * a_norm),
    cos = (sum_r E*A) / (||cap|| * sqrt(E^T G E)) since the softmax
    normalizer Z cancels between numerator and denominator.
  - wei-norm uses the per-image Gram matrix G_i = X_i X_i^T; the
    blockdiag-masked 3-image Gram blocks are host-precomputed (they are
    pure input preprocessing, like the transposed/padded image layout).
  - word masking is folded into the caption operand host-side; caption
    word norms ||cap_w||^2 (cn2) are host-precomputed as well.
  - all matmul operands are bf16 (validated: end-to-end rel err ~5e-5);
    PSUM accumulation stays fp32.
  - the raw attention A is consumed immediately (Prelu -> B, inverse
    Prelu B -> A2 = min(10B, B)), so each attention PSUM bank is freed
    after one ACT pass and the pipeline can run 3 groups deep per pool.
"""

import numpy as np

# Problem geometry (hardcoded per contract).
I, R, D, W = 128, 36, 512, 24
NCORES = 8
CS = I // NCORES          # captions per core = 16
GI = 3                    # images per PE group (3*36 = 108 <= 128 partitions)
GR = GI * R               # 108
NG = (I + GI - 1) // GI   # 43 groups
SW = 128                  # stationary width: pad each group's imT slice to
                          # 128 columns so LDWEIGHTS takes the FWL fast path
IRP = (NG - 1) * GR + SW  # 4664 padded image-region columns
CW = CS * W               # 384 caption-word columns per core
NK = D // 128             # 4 contraction chunks
LN9 = float(np.log(9.0))

_CACHE: dict = {}


def _build_program(reps: int = 1, with_loss: bool = True):
    import concourse.bacc as bacc
    import concourse.mybir as mybir
    import concourse.tile as tile

    f32 = mybir.dt.float32
    bf16 = mybir.dt.bfloat16
    Act = mybir.ActivationFunctionType
    Alu = mybir.AluOpType
    X = mybir.AxisListType.X

    nc = bacc.Bacc("TRN2", target_bir_lowering=False, debug=False,
                   num_devices=NCORES)

    imT_d = nc.dram_tensor("imT", [NK, 128, IRP], bf16, kind="ExternalInput")
    capT_d = nc.dram_tensor("capT", [NK, 128, CW], bf16, kind="ExternalInput")
    gsb_d = nc.dram_tensor("gsb", [GR, NG * GR], bf16, kind="ExternalInput")
    cn2_d = nc.dram_tensor("cn2", [128, CW], f32, kind="ExternalInput")
    wmask_d = nc.dram_tensor("wmask", [128, CW], f32, kind="ExternalInput")
    inds_d = nc.dram_tensor("inds", [GR, NG * 128], bf16, kind="ExternalInput")
    sc_d = nc.dram_tensor("scores", [128, CS], f32, kind="ExternalOutput")

    with tile.TileContext(nc) as tc:
        with (
            tc.tile_pool(name="const", bufs=1) as cp,
            tc.tile_pool(name="work", bufs=6) as wp,
            tc.tile_pool(name="small", bufs=8) as sp,
        ):
            # ---- constants -------------------------------------------------
            capT = [cp.tile([128, CW], bf16, tag=f"capT{k}", name=f"capT{k}")
                    for k in range(NK)]
            gsb = cp.tile([GR, NG * GR], bf16, tag="gsb")
            cn2 = cp.tile([128, CW], f32, tag="cn2")
            wmask = cp.tile([128, CW], f32, tag="wmask")
            inds = cp.tile([GR, NG * 128], bf16, tag="inds")
            scores = cp.tile([128, CS], f32, tag="scores")
            ln9c = cp.tile([128, 1], f32, tag="ln9c")
            nc.gpsimd.memset(ln9c[:], LN9)
            imTsb = [cp.tile([128, IRP], bf16, tag=f"imT{k}", name=f"imT{k}")
                     for k in range(NK)]

            for k in range(NK):
                nc.sync.dma_start(out=capT[k][:], in_=capT_d[k])
            nc.sync.dma_start(out=cn2[:], in_=cn2_d[:])
            nc.sync.dma_start(out=wmask[:], in_=wmask_d[:])
            nc.sync.dma_start(out=inds[:], in_=inds_d[:])
            nc.sync.dma_start(out=gsb[:], in_=gsb_d[:])
            # images resident in SBUF; column-block DMAs (~7 groups each),
            # all 4 contraction chunks of a column block before the next
            # block, so the first groups unblock as early as possible
            CB = 7 * GR
            for c0 in range(0, IRP, CB):
                c1 = min(c0 + CB, IRP)
                for k in range(NK):
                    nc.sync.dma_start(
                        out=imTsb[k][:, c0:c1],
                        in_=imT_d[k, :, c0:c1],
                    )

            NP = NG // 2  # group pairs; NG=43 is odd -> last pair is single
            for rep in range(reps):
                with tc.tile_pool(name=f"ps_acc{rep}", bufs=1,
                                  space="PSUM") as pa:
                    ps_ne = pa.tile([128, CW], f32, tag="ne", name="ps_ne")
                    ps_q = pa.tile([128, CW], f32, tag="q", name="ps_q")

                    with (
                        tc.tile_pool(name=f"ps_a{rep}", bufs=2,
                                     space="PSUM") as pta,
                        tc.tile_pool(name=f"ps_g{rep}", bufs=1,
                                     space="PSUM") as ptg,
                    ):
                      for p in range(NP + 1):
                        gs = [2 * p] if p == NP else [2 * p, 2 * p + 1]
                        U = len(gs)
                        UW = U * CW
                        # 2-bank PSUM tile: A(g0) [0:384], A(g1) [512:896]
                        pa2 = pta.tile([GR, 512 * U], f32, tag="a",
                                       name="pa2")
                        for u, g in enumerate(gs):
                            for k in range(NK):
                                sl = imTsb[k][:, g * GR:(g + 1) * GR]
                                nc.tensor.matmul(
                                    pa2[:, 512 * u:512 * u + CW],
                                    sl, capT[k][:],
                                    start=(k == 0), stop=(k == NK - 1))
                        av = (pa2[:].rearrange("p (u w) -> p u w", u=U)
                              [:, :, 0:CW])
                        # B = leaky_relu(A), one batched ACT pass per pair;
                        # frees both attention PSUM banks
                        B = wp.tile([GR, UW], bf16, tag="B", name="B")
                        nc.scalar.activation(
                            B[:].rearrange("p (u w) -> p u w", u=U), av,
                            Act.Prelu, alpha=0.1)
                        # A2 = min(10B, B) inverts the leaky-relu (DVE: the
                        # backend rejects TensorScalarPtr on Pool)
                        A2 = wp.tile([GR, UW], bf16, tag="A2", name="A2")
                        nc.vector.scalar_tensor_tensor(
                            A2[:], B[:], 10.0, B[:], Alu.mult, Alu.min)
                        # n2 = sum_w B^2 per (row, caption)
                        B2 = wp.tile([GR, UW], bf16, tag="B2", name="B2")
                        nc.gpsimd.tensor_tensor(B2[:], B[:], B[:], Alu.mult)
                        n2 = sp.tile([GR, U * CS], f32, tag="n2", name="n2")
                        nc.vector.reduce_sum(
                            n2[:], B2[:].rearrange("p (c w) -> p c w", w=W),
                            axis=X,
                        )
                        # rinv9 = 9 * n2^(-1/2) via exp(-0.5*ln + ln9)
                        lnn = sp.tile([GR, U * CS], f32, tag="lnn",
                                      name="lnn")
                        nc.scalar.activation(lnn[:], n2[:], Act.Ln)
                        rinv9 = sp.tile([GR, U * CS], bf16, tag="rinv9",
                                        name="rinv9")
                        nc.scalar.activation(rinv9[:], lnn[:], Act.Exp,
                                             scale=-0.5, bias=ln9c[0:GR])
                        # Bn9 = 9 * B * rinv (broadcast over words)
                        Bn9 = wp.tile([GR, UW], bf16, tag="Bn9", name="Bn9")
                        nc.gpsimd.tensor_tensor(
                            Bn9[:].rearrange("p (c w) -> p c w", w=W),
                            B[:].rearrange("p (c w) -> p c w", w=W),
                            rinv9[:].rearrange("p (c u) -> p c u", u=1)
                            .broadcast_to((GR, U * CS, W)),
                            Alu.mult,
                        )
                        # E = exp(Bn9), one batched ACT pass per pair
                        E = wp.tile([GR, UW], bf16, tag="E", name="E")
                        nc.scalar.activation(E[:], Bn9[:], Act.Exp)
                        # GE = blockdiag(G) @ E per group, into a 2-bank tile
                        pg2 = ptg.tile([GR, 512 * U], f32, tag="ge",
                                       name="pg2")
                        for u, g in enumerate(gs):
                            nc.tensor.matmul(
                                pg2[:, 512 * u:512 * u + CW],
                                gsb[:, g * GR:(g + 1) * GR],
                                E[:, u * CW:(u + 1) * CW],
                                start=True, stop=True)
                        # EA = E * A2 (batched, Pool) / EGE = E * GE (DVE)
                        EA = wp.tile([GR, UW], bf16, tag="EA", name="EA")
                        nc.gpsimd.tensor_tensor(EA[:], E[:], A2[:], Alu.mult)
                        EG = wp.tile([GR, UW], bf16, tag="EG", name="EG")
                        nc.vector.tensor_tensor(
                            EG[:].rearrange("p (u w) -> p u w", u=U),
                            E[:].rearrange("p (u w) -> p u w", u=U),
                            pg2[:].rearrange("p (u w) -> p u w", u=U)
                            [:, :, 0:CW],
                            Alu.mult,
                        )
                        # block-sum over regions into stacked accumulators
                        for u, g in enumerate(gs):
                            ind = inds[:, g * 128:(g + 1) * 128]
                            cw0, cw1 = u * CW, (u + 1) * CW
                            nc.tensor.matmul(
                                ps_ne[:], ind, EA[:, cw0:cw1],
                                start=(g == 0), stop=(g == NG - 1))
                            nc.tensor.matmul(
                                ps_q[:], ind, EG[:, cw0:cw1],
                                start=(g == 0), stop=(g == NG - 1))

                    # ---- epilogue: cos -> logsumexp ------------------------
                    with tc.tile_pool(name=f"fin{rep}", bufs=1) as fp_:
                        qc = fp_.tile([128, CW], f32, tag="qc", name="qc")
                        nc.vector.tensor_tensor(qc[:], ps_q[:], cn2[:],
                                                Alu.mult)
                        lq = fp_.tile([128, CW], f32, tag="lq", name="lq")
                        nc.scalar.activation(lq[:], qc[:], Act.Ln)
                        rsq = fp_.tile([128, CW], f32, tag="rsq", name="rsq")
                        nc.scalar.activation(rsq[:], lq[:], Act.Exp,
                                             scale=-0.5)
                        cosm = fp_.tile([128, CW], f32, tag="cosm",
                                        name="cosm")
                        nc.vector.tensor_tensor(cosm[:], ps_ne[:], rsq[:],
                                                Alu.mult)
                        ex = fp_.tile([128, CW], f32, tag="ex", name="ex")
                        nc.scalar.activation(ex[:], cosm[:], Act.Exp,
                                             scale=6.0)
                        exm = fp_.tile([128, CW], f32, tag="exm", name="exm")
                        nc.vector.tensor_tensor(exm[:], ex[:], wmask[:],
                                                Alu.mult)
                        rs = fp_.tile([128, CS], f32, tag="rs", name="rs")
                        nc.vector.reduce_sum(
                            rs[:], exm[:].rearrange("p (c w) -> p c w", w=W),
                            axis=X,
                        )
                        # scores (x6): L = ln(sum) = 6 * row_sim
                        nc.scalar.activation(scores[:], rs[:], Act.Ln)
                        if rep == reps - 1:
                            nc.sync.dma_start(out=sc_d[:], in_=scores[:])

    # Pin activation-table selection to the one set that contains every
    # scalar-engine function we use (prelu, ln, exp): otherwise the
    # inserter alternates sets and pays a 1.3us table load per switch.
    from concourse import bacc as _bacc_mod
    _orig_tables = _bacc_mod.get_activation_tables

    def _pinned_tables(arch):
        t = _orig_tables(arch)
        keep = "natural_log_exp_and_others"
        return {k: (v if k == keep else set()) for k, v in t.items()}

    _bacc_mod.get_activation_tables = _pinned_tables
    try:
        nc.compile()
    finally:
        _bacc_mod.get_activation_tables = _orig_tables
    return nc


def _prep_in_maps(images, captions, cap_lens):
    import ml_dtypes

    bf = ml_dtypes.bfloat16
    images = np.ascontiguousarray(images, dtype=np.float32)
    captions = np.ascontiguousarray(captions, dtype=np.float32)
    cap_lens = np.asarray(cap_lens, dtype=np.int32)

    imt = images.transpose(2, 0, 1).reshape(D, I * R)
    # pad the ragged last group with a dummy (real-valued) region block
    imt_p = np.concatenate([imt, imt[:, : IRP - I * R]], axis=1)
    imt_p = np.ascontiguousarray(imt_p).astype(bf).reshape(NK, 128, IRP)

    # per-image Gram matrices from the bf16-rounded images (matches the
    # on-device attention operand rounding), assembled as the blockdiag
    # 3-image stationary blocks per group
    imb = images.astype(bf).astype(np.float32)          # [I, R, D]
    G = np.matmul(imb, imb.transpose(0, 2, 1))          # [I, R, R]
    gsb = np.zeros((GR, NG * GR), dtype=bf)
    for g in range(NG):
        for b in range(GI):
            m = GI * g + b
            if m >= I:
                m = 0  # dummy pad image (matches imt_p padding)
            gsb[b * R:(b + 1) * R,
                g * GR + b * R:g * GR + (b + 1) * R] = G[m]

    inds = np.zeros((GR, NG * 128), dtype=bf)
    for g in range(NG):
        for k in range(GR):
            m = GI * g + k // R
            if m < I:
                inds[k, g * 128 + m] = 1.0

    wvalid = (np.arange(W)[None, :] < cap_lens[:, None]).astype(np.float32)
    capm = captions * wvalid[:, :, None]              # masked words zeroed
    cn2_full = np.sum(captions * captions, axis=2)    # [I, W] unmasked norms

    in_maps = []
    for r in range(NCORES):
        cap = capm[r * CS:(r + 1) * CS]                  # [16, 24, 512]
        capT = np.ascontiguousarray(
            cap.transpose(2, 0, 1).reshape(D, CW)
        ).astype(bf).reshape(NK, 128, CW)
        wm = np.ascontiguousarray(np.broadcast_to(
            wvalid[r * CS:(r + 1) * CS].reshape(1, CW), (128, CW)
        )).astype(np.float32)
        cn2 = np.ascontiguousarray(np.broadcast_to(
            cn2_full[r * CS:(r + 1) * CS].reshape(1, CW), (128, CW)
        )).astype(np.float32)
        in_maps.append({
            "imT": imt_p,
            "capT": capT,
            "gsb": gsb,
            "cn2": cn2,
            "wmask": wm,
            "inds": inds,
        })
    return in_maps


def _get_nc(reps: int = 1, with_loss: bool = True):
    key = (reps, with_loss)
    if key not in _CACHE:
        _CACHE[key] = _build_program(reps, with_loss)
    return _CACHE[key]


def _host_loss(scores_x6: np.ndarray) -> np.float32:
    """scores_x6: [I, C] = 6 * row_sim. Diagonal-margin loss in fp32."""
    s = (scores_x6 / 6.0).astype(np.float32)
    diag = np.diag(s)
    cost_s = np.clip(0.2 + s - diag[:, None], 0.0, None)
    cost_im = np.clip(0.2 + s - diag[None, :], 0.0, None)
    np.fill_diagonal(cost_s, 0.0)
    np.fill_diagonal(cost_im, 0.0)
    return np.float32(cost_s.max(axis=1).sum() + cost_im.max(axis=0).sum())


def kernel(images, captions, cap_lens):
    from concourse.bass_utils import run_bass_kernel_spmd

    nc = _get_nc()
    in_maps = _prep_in_maps(images, captions, cap_lens)
    res = run_bass_kernel_spmd(nc, in_maps, core_ids=list(range(NCORES)))
    blocks = [np.asarray(res.results[r]["scores"]) for r in range(NCORES)]
    scores_x6 = np.concatenate(blocks, axis=1)        # [128, 128]
    return _host_loss(scores_x6)


# revision 17
# speedup vs baseline: 5.9800x; 2.1849x over previous
"""Trainium2 Bass kernel for nn_ContrastiveLoss (SCAN text-to-image loss).

Full inputs in, full (scalar) output out. Captions are sharded across 8
NeuronCores (16 each), balanced by length: every core gets the same number
of short and long captions, and caption words are packed into two
fixed-width buckets (8 captions x WS slots + 8 x WL slots, WS/WL = the
global per-bucket maximum length). That shrinks the free dimension from
16*24=384 padded columns to ~8*14+8*24=304, cutting matmul cycles and
elementwise work by ~20%. Each core computes its scores block on device;
the host scatters the 8 slices back to original caption order and computes
the tiny [128,128] diagonal-margin loss in numpy.

Math notes (exact reductions of the reference):
  - softmax over regions needs no normalizer: with E = exp(9 * a_norm),
    cos = (sum_r E*A) / (||cap|| * sqrt(E^T G E)) since the softmax
    normalizer Z cancels between numerator and denominator.
  - wei-norm uses the per-image Gram matrix G_i = X_i X_i^T; the
    blockdiag-masked 3-image Gram blocks are host-precomputed (input
    preprocessing, like the transposed/padded image layout).
  - word masking is folded into the caption operand host-side; caption
    word norms ||cap_w||^2 (cn2) are host-precomputed as well.
  - all matmul operands are bf16 (validated: end-to-end rel err ~5e-5);
    PSUM accumulation stays fp32.
  - the raw attention A is consumed immediately (Prelu -> B, inverse
    Prelu B -> A2 = min(10B, B) on DVE), so each attention PSUM bank is
    freed after one ACT pass and the pipeline runs deep.
"""

import numpy as np

# Problem geometry (hardcoded per contract).
I, R, D, W = 128, 36, 512, 24
NCORES = 8
CS = I // NCORES          # captions per core = 16
NB = CS // 2              # captions per length bucket per core = 8
GI = 3                    # images per PE group (3*36 = 108 <= 128 partitions)
GR = GI * R               # 108
NG = (I + GI - 1) // GI   # 43 groups
SW = 128                  # stationary width: pad each group's imT slice to
                          # 128 columns so LDWEIGHTS takes the FWL fast path
IRP = (NG - 1) * GR + SW  # 4664 padded image-region columns
NK = D // 128             # 4 contraction chunks
LN9 = float(np.log(9.0))

_CACHE: dict = {}
_LAYOUT: dict = {}        # ws, wl, sel — set by _prep_in_maps


def _build_program(reps: int = 1, ws: int = 14, wl: int = 24,
                   with_loss: bool = True):
    import concourse.bacc as bacc
    import concourse.mybir as mybir
    import concourse.tile as tile

    f32 = mybir.dt.float32
    bf16 = mybir.dt.bfloat16
    Act = mybir.ActivationFunctionType
    Alu = mybir.AluOpType
    X = mybir.AxisListType.X

    cwp = NB * ws + NB * wl                 # packed caption-word columns
    assert cwp <= 512
    buckets = [(0, 0, NB, ws), (NB * ws, NB, NB, wl)]  # (col0, cap0, n, w)

    nc = bacc.Bacc("TRN2", target_bir_lowering=False, debug=False,
                   num_devices=NCORES)

    imT_d = nc.dram_tensor("imT", [NK, 128, IRP], bf16, kind="ExternalInput")
    capT_d = nc.dram_tensor("capT", [NK, 128, cwp], bf16,
                            kind="ExternalInput")
    gsb_d = nc.dram_tensor("gsb", [GR, NG * GR], bf16, kind="ExternalInput")
    cn2_d = nc.dram_tensor("cn2", [128, cwp], f32, kind="ExternalInput")
    wmask_d = nc.dram_tensor("wmask", [128, cwp], f32, kind="ExternalInput")
    inds_d = nc.dram_tensor("inds", [GR, NG * 128], bf16,
                            kind="ExternalInput")
    sc_d = nc.dram_tensor("scores", [128, CS], f32, kind="ExternalOutput")

    with tile.TileContext(nc) as tc:
        with (
            tc.tile_pool(name="const", bufs=1) as cp,
            tc.tile_pool(name="work", bufs=6) as wp,
            tc.tile_pool(name="small", bufs=8) as sp,
        ):
            # ---- constants -------------------------------------------------
            capT = [cp.tile([128, cwp], bf16, tag=f"capT{k}", name=f"capT{k}")
                    for k in range(NK)]
            gsb = cp.tile([GR, NG * GR], bf16, tag="gsb")
            cn2 = cp.tile([128, cwp], f32, tag="cn2")
            wmask = cp.tile([128, cwp], f32, tag="wmask")
            inds = cp.tile([GR, NG * 128], bf16, tag="inds")
            scores = cp.tile([128, CS], f32, tag="scores")
            ln9c = cp.tile([128, 1], f32, tag="ln9c")
            nc.gpsimd.memset(ln9c[:], LN9)
            imTsb = [cp.tile([128, IRP], bf16, tag=f"imT{k}", name=f"imT{k}")
                     for k in range(NK)]

            for k in range(NK):
                nc.sync.dma_start(out=capT[k][:], in_=capT_d[k])
            nc.sync.dma_start(out=cn2[:], in_=cn2_d[:])
            nc.sync.dma_start(out=wmask[:], in_=wmask_d[:])
            nc.sync.dma_start(out=inds[:], in_=inds_d[:])
            nc.sync.dma_start(out=gsb[:], in_=gsb_d[:])
            # images resident in SBUF; column-block DMAs (~7 groups each),
            # all 4 contraction chunks of a column block before the next
            # block, so the first groups unblock as early as possible
            CB = 7 * GR
            for c0 in range(0, IRP, CB):
                c1 = min(c0 + CB, IRP)
                for k in range(NK):
                    nc.sync.dma_start(
                        out=imTsb[k][:, c0:c1],
                        in_=imT_d[k, :, c0:c1],
                    )

            NP = NG // 2  # group pairs; NG=43 is odd -> last pair is single
            for rep in range(reps):
                with tc.tile_pool(name=f"ps_acc{rep}", bufs=1,
                                  space="PSUM") as pa:
                    ps_ne = pa.tile([128, cwp], f32, tag="ne", name="ps_ne")
                    ps_q = pa.tile([128, cwp], f32, tag="q", name="ps_q")

                    with (
                        tc.tile_pool(name=f"ps_a{rep}", bufs=2,
                                     space="PSUM") as pta,
                        tc.tile_pool(name=f"ps_g{rep}", bufs=1,
                                     space="PSUM") as ptg,
                    ):
                      for p in range(NP + 1):
                        gs = [2 * p] if p == NP else [2 * p, 2 * p + 1]
                        U = len(gs)
                        UW = U * cwp
                        # 2-bank PSUM tile: A(g0) [0:cwp], A(g1) [512:...]
                        pa2 = pta.tile([GR, 512 * U], f32, tag="a",
                                       name="pa2")
                        for u, g in enumerate(gs):
                            for k in range(NK):
                                sl = imTsb[k][:, g * GR:(g + 1) * GR]
                                nc.tensor.matmul(
                                    pa2[:, 512 * u:512 * u + cwp],
                                    sl, capT[k][:],
                                    start=(k == 0), stop=(k == NK - 1))
                        av = (pa2[:].rearrange("p (u w) -> p u w", u=U)
                              [:, :, 0:cwp])
                        # B = leaky_relu(A), one batched ACT pass per pair;
                        # frees both attention PSUM banks
                        B = wp.tile([GR, UW], bf16, tag="B", name="B")
                        nc.scalar.activation(
                            B[:].rearrange("p (u w) -> p u w", u=U), av,
                            Act.Prelu, alpha=0.1)
                        # A2 = min(10B, B) inverts the leaky-relu (DVE: the
                        # backend rejects TensorScalarPtr on Pool)
                        A2 = wp.tile([GR, UW], bf16, tag="A2", name="A2")
                        nc.vector.scalar_tensor_tensor(
                            A2[:], B[:], 10.0, B[:], Alu.mult, Alu.min)
                        # n2 = sum_w B^2 per (row, caption), one reduce per
                        # word-width bucket
                        B2 = wp.tile([GR, UW], bf16, tag="B2", name="B2")
                        nc.gpsimd.tensor_tensor(B2[:], B[:], B[:], Alu.mult)
                        n2 = sp.tile([GR, U * CS], f32, tag="n2", name="n2")
                        for col0, cap0, nb, w_ in buckets:
                            nc.vector.reduce_sum(
                                n2[:].rearrange("p (u c) -> p u c", u=U)
                                [:, :, cap0:cap0 + nb],
                                B2[:].rearrange("p (u q) -> p u q", u=U)
                                [:, :, col0:col0 + nb * w_]
                                .rearrange("p u (c w) -> p u c w", w=w_),
                                axis=X,
                            )
                        # rinv9 = 9 * n2^(-1/2) via exp(-0.5*ln + ln9)
                        lnn = sp.tile([GR, U * CS], f32, tag="lnn",
                                      name="lnn")
                        nc.scalar.activation(lnn[:], n2[:], Act.Ln)
                        rinv9 = sp.tile([GR, U * CS], bf16, tag="rinv9",
                                        name="rinv9")
                        nc.scalar.activation(rinv9[:], lnn[:], Act.Exp,
                                             scale=-0.5, bias=ln9c[0:GR])
                        # Bn9 = 9 * B * rinv (broadcast over words), one op
                        # per bucket
                        Bn9 = wp.tile([GR, UW], bf16, tag="Bn9", name="Bn9")
                        for col0, cap0, nb, w_ in buckets:
                            nc.gpsimd.tensor_tensor(
                                Bn9[:].rearrange("p (u q) -> p u q", u=U)
                                [:, :, col0:col0 + nb * w_]
                                .rearrange("p u (c w) -> p u c w", w=w_),
                                B[:].rearrange("p (u q) -> p u q", u=U)
                                [:, :, col0:col0 + nb * w_]
                                .rearrange("p u (c w) -> p u c w", w=w_),
                                rinv9[:].rearrange("p (u c) -> p u c", u=U)
                                [:, :, cap0:cap0 + nb]
                                .rearrange("p u (c o) -> p u c o", o=1)
                                .broadcast_to((GR, U, nb, w_)),
                                Alu.mult,
                            )
                        # E = exp(Bn9), one batched ACT pass per pair
                        E = wp.tile([GR, UW], bf16, tag="E", name="E")
                        nc.scalar.activation(E[:], Bn9[:], Act.Exp)
                        # GE = blockdiag(G) @ E per group, into a 2-bank tile
                        pg2 = ptg.tile([GR, 512 * U], f32, tag="ge",
                                       name="pg2")
                        for u, g in enumerate(gs):
                            nc.tensor.matmul(
                                pg2[:, 512 * u:512 * u + cwp],
                                gsb[:, g * GR:(g + 1) * GR],
                                E[:, u * cwp:(u + 1) * cwp],
                                start=True, stop=True)
                        # EA = E * A2 (batched, Pool) / EGE = E * GE (DVE)
                        EA = wp.tile([GR, UW], bf16, tag="EA", name="EA")
                        nc.gpsimd.tensor_tensor(EA[:], E[:], A2[:], Alu.mult)
                        EG = wp.tile([GR, UW], bf16, tag="EG", name="EG")
                        nc.vector.tensor_tensor(
                            EG[:].rearrange("p (u w) -> p u w", u=U),
                            E[:].rearrange("p (u w) -> p u w", u=U),
                            pg2[:].rearrange("p (u w) -> p u w", u=U)
                            [:, :, 0:cwp],
                            Alu.mult,
                        )
                        # block-sum over regions into stacked accumulators
                        for u, g in enumerate(gs):
                            ind = inds[:, g * 128:(g + 1) * 128]
                            cw0, cw1 = u * cwp, (u + 1) * cwp
                            nc.tensor.matmul(
                                ps_ne[:], ind, EA[:, cw0:cw1],
                                start=(g == 0), stop=(g == NG - 1))
                            nc.tensor.matmul(
                                ps_q[:], ind, EG[:, cw0:cw1],
                                start=(g == 0), stop=(g == NG - 1))

                    # ---- epilogue: cos -> logsumexp ------------------------
                    with tc.tile_pool(name=f"fin{rep}", bufs=1) as fp_:
                        qc = fp_.tile([128, cwp], f32, tag="qc", name="qc")
                        nc.vector.tensor_tensor(qc[:], ps_q[:], cn2[:],
                                                Alu.mult)
                        lq = fp_.tile([128, cwp], f32, tag="lq", name="lq")
                        nc.scalar.activation(lq[:], qc[:], Act.Ln)
                        rsq = fp_.tile([128, cwp], f32, tag="rsq",
                                       name="rsq")
                        nc.scalar.activation(rsq[:], lq[:], Act.Exp,
                                             scale=-0.5)
                        cosm = fp_.tile([128, cwp], f32, tag="cosm",
                                        name="cosm")
                        nc.vector.tensor_tensor(cosm[:], ps_ne[:], rsq[:],
                                                Alu.mult)
                        ex = fp_.tile([128, cwp], f32, tag="ex", name="ex")
                        nc.scalar.activation(ex[:], cosm[:], Act.Exp,
                                             scale=6.0)
                        exm = fp_.tile([128, cwp], f32, tag="exm",
                                       name="exm")
                        nc.vector.tensor_tensor(exm[:], ex[:], wmask[:],
                                                Alu.mult)
                        rs = fp_.tile([128, CS], f32, tag="rs", name="rs")
                        for col0, cap0, nb, w_ in buckets:
                            nc.vector.reduce_sum(
                                rs[:, cap0:cap0 + nb],
                                exm[:, col0:col0 + nb * w_]
                                .rearrange("p (c w) -> p c w", w=w_),
                                axis=X,
                            )
                        # scores (x6): L = ln(sum) = 6 * row_sim
                        nc.scalar.activation(scores[:], rs[:], Act.Ln)
                        if rep == reps - 1:
                            nc.sync.dma_start(out=sc_d[:], in_=scores[:])

    # Pin activation-table selection to the one set that contains every
    # scalar-engine function we use (prelu, ln, exp): otherwise the
    # inserter alternates sets and pays a 1.3us table load per switch.
    from concourse import bacc as _bacc_mod
    _orig_tables = _bacc_mod.get_activation_tables

    def _pinned_tables(arch):
        t = _orig_tables(arch)
        keep = "natural_log_exp_and_others"
        return {k: (v if k == keep else set()) for k, v in t.items()}

    _bacc_mod.get_activation_tables = _pinned_tables
    try:
        nc.compile()
    finally:
        _bacc_mod.get_activation_tables = _orig_tables
    return nc


def _prep_in_maps(images, captions, cap_lens):
    import ml_dtypes

    bf = ml_dtypes.bfloat16
    images = np.ascontiguousarray(images, dtype=np.float32)
    captions = np.ascontiguousarray(captions, dtype=np.float32)
    cap_lens = np.asarray(cap_lens, dtype=np.int32)

    # length-balanced caption sharding: 8 short + 8 long per core, packed
    # to per-bucket global max widths
    order = np.argsort(cap_lens, kind="stable")
    short, long_ = order[:NCORES * NB], order[NCORES * NB:]
    ws = max(int(cap_lens[short].max()), 2)
    wl = max(int(cap_lens[long_].max()), 2)
    cwp = NB * ws + NB * wl
    sel = [np.concatenate([short[r * NB:(r + 1) * NB],
                           long_[r * NB:(r + 1) * NB]])
           for r in range(NCORES)]
    _LAYOUT["ws"], _LAYOUT["wl"], _LAYOUT["sel"] = ws, wl, sel

    imt = images.transpose(2, 0, 1).reshape(D, I * R)
    # pad the ragged last group with a dummy (real-valued) region block
    imt_p = np.concatenate([imt, imt[:, : IRP - I * R]], axis=1)
    imt_p = np.ascontiguousarray(imt_p).astype(bf).reshape(NK, 128, IRP)

    # per-image Gram matrices from the bf16-rounded images (matches the
    # on-device attention operand rounding), assembled as the blockdiag
    # 3-image stationary blocks per group
    imb = images.astype(bf).astype(np.float32)          # [I, R, D]
    G = np.matmul(imb, imb.transpose(0, 2, 1))          # [I, R, R]
    gsb = np.zeros((GR, NG * GR), dtype=bf)
    for g in range(NG):
        for b in range(GI):
            m = GI * g + b
            if m >= I:
                m = 0  # dummy pad image (matches imt_p padding)
            gsb[b * R:(b + 1) * R,
                g * GR + b * R:g * GR + (b + 1) * R] = G[m]

    inds = np.zeros((GR, NG * 128), dtype=bf)
    for g in range(NG):
        for k in range(GR):
            m = GI * g + k // R
            if m < I:
                inds[k, g * 128 + m] = 1.0

    in_maps = []
    for r in range(NCORES):
        capw = np.zeros((cwp, D), dtype=np.float32)
        wm = np.zeros((cwp,), dtype=np.float32)
        c2 = np.ones((cwp,), dtype=np.float32)
        off = 0
        for j, c in enumerate(sel[r]):
            w_ = ws if j < NB else wl
            L = int(cap_lens[c])
            capw[off:off + L] = captions[c, :L]
            wm[off:off + L] = 1.0
            # unmasked norms for every slot (masked slots need a finite,
            # nonzero denominator; their scores are zeroed by wmask)
            c2[off:off + w_] = np.sum(captions[c, :w_] ** 2, axis=1)
            off += w_
        capT = np.ascontiguousarray(capw.T).astype(bf).reshape(NK, 128, cwp)
        in_maps.append({
            "imT": imt_p,
            "capT": capT,
            "gsb": gsb,
            "cn2": np.ascontiguousarray(
                np.broadcast_to(c2[None, :], (128, cwp))),
            "wmask": np.ascontiguousarray(
                np.broadcast_to(wm[None, :], (128, cwp))),
            "inds": inds,
        })
    return in_maps


def _get_nc(reps: int = 1, with_loss: bool = True):
    assert _LAYOUT, "_prep_in_maps must run before _get_nc"
    key = (reps, _LAYOUT["ws"], _LAYOUT["wl"])
    if key not in _CACHE:
        _CACHE[key] = _build_program(reps, _LAYOUT["ws"], _LAYOUT["wl"])
    return _CACHE[key]


def _host_loss(scores_x6: np.ndarray) -> np.float32:
    """scores_x6: [I, C] = 6 * row_sim. Diagonal-margin loss in fp32."""
    s = (scores_x6 / 6.0).astype(np.float32)
    diag = np.diag(s)
    cost_s = np.clip(0.2 + s - diag[:, None], 0.0, None)
    cost_im = np.clip(0.2 + s - diag[None, :], 0.0, None)
    np.fill_diagonal(cost_s, 0.0)
    np.fill_diagonal(cost_im, 0.0)
    return np.float32(cost_s.max(axis=1).sum() + cost_im.max(axis=0).sum())


def kernel(images, captions, cap_lens):
    from concourse.bass_utils import run_bass_kernel_spmd

    in_maps = _prep_in_maps(images, captions, cap_lens)
    nc = _get_nc()
    res = run_bass_kernel_spmd(nc, in_maps, core_ids=list(range(NCORES)))
    scores_x6 = np.zeros((I, I), dtype=np.float32)
    for r in range(NCORES):
        blk = np.asarray(res.results[r]["scores"])     # [128, 16]
        for j, c in enumerate(_LAYOUT["sel"][r]):
            scores_x6[:, c] = blk[:, j]
    return _host_loss(scores_x6)
